# revision 42
# baseline (speedup 1.0000x reference)
"""Trainium2 Bass kernel for the NSDE model (Euler-Maruyama scan + MLPs).

Strategy:
  - Data-parallel over batch: 16384 rows -> 8 cores x 2048 rows.
  - Only the 20 time slices of x_path that the scan actually reads are
    shipped to the device (indices computed on host from t_span).
  - Feature-major layout on chip: activations are [feature, batch] so every
    matmul uses the weight matrix directly as lhsT (out = W^T @ actT) and
    biases are per-partition scalars.
  - The per-core batch (2048) is processed as TWO independent interleaved
    streams of 1024 rows; 64-feature tensors are "packed": partitions 0-63
    hold features of a stream's first 512 rows, partitions 64-127 the
    second 512.
  - Everything runs in fp16 (weights, x, noise, and the h carry): fp16
    matmuls stream 1 col/cycle at full clock (f32r runs at half clock) and
    fp16's 11-bit mantissa keeps the 20-step carry accurate (~1e-3 final
    rel err, validated against the reference in float simulation).
  - The h carry is added into the drift-out PSUM exactly via an fp16
    identity matmul (1.0 * h is exact), so h' = (psum3 + dt*db3) + sig*zs
    is a single fused scalar_tensor_tensor per stream.
  - Engine queues are strict FIFO, so per-step instructions are issued in
    dependency-arrival order: stream 1 trails stream 0, and every engine's
    queue lists all stream-0 work for a stage before stream-1 work that
    becomes ready later (avoids head-of-line blocking).
  - Elementwise split per stream: ACT does relu/sigmoid halves, DVE does
    the other halves + the fused h-update, GpSimd does the noise multiply
    (the only SBUF-only op -- GpSimd has no PSUM port).
  - dt and sqrt(dt) folds on host: dW3*dt per step, db3*dt, noise
    pre-scaled zs = dW * gscale * sqrt(dt). Step 0 is specialized (h=0):
    no diffusion net, no w1h matmuls, and sigmoid(const) is folded into
    zs[0] on the host.
  - Prologue: a tiny garbage tile plus the step-0 weight slice (w1x, w2,
    dt0*w3) are DMA'd before the bulk consts so the first drift matmuls
    and the PE warm-up start as early as possible.
"""

import os
from contextlib import ExitStack

import numpy as np

import concourse.bass as bass
import concourse.mybir as mybir
import concourse.tile as tile
from concourse import bacc
from concourse.bass_utils import run_bass_kernel_spmd

F32 = mybir.dt.float32
F16 = mybir.dt.float16
AF = mybir.ActivationFunctionType
ALU = mybir.AluOpType

NPF16 = np.float16

STEPS = 20
NCORES = 8
B = 16384
BC = B // NCORES  # per-core batch: 2048
SB = BC // 2  # per-stream batch: 1024
HB = SB // 2  # packed free size per stream: 512
H = 64
FX = 64
DW = 128

# cw_early columns: w1x | w2 | w3s[0]
CWE_COLS = 128 + 128 + 64  # 320
# cw_main columns: w1h | w3s[1:20] | gw1 | gw2 | idn | rw1 | rw2
W1H_OF = 0
W3S_OF = 128  # steps 1..19 -> 128 + 64*(k-1)
GW1_OF = W3S_OF + (STEPS - 1) * H  # 1344
GW2_OF = GW1_OF + 128  # 1472
IDN_OF = GW2_OF + 128  # 1600
RW1_OF = IDN_OF + 128  # 1728
RW2_OF = RW1_OF + 32  # 1760
CWM_COLS = RW2_OF + 2  # 1762

_CACHE = {}


def _build():
    if "nc" in _CACHE:
        return _CACHE["nc"]

    nc = bacc.Bacc("TRN2", target_bir_lowering=False, debug=False)

    d_xt = nc.dram_tensor("xt", [STEPS, 128, 2, HB], F16, kind="ExternalInput")
    d_zst = nc.dram_tensor("zst", [STEPS, 128, 2, HB], F16, kind="ExternalInput")
    # step-0 bundle: cwe weights | xt[0] in one DMA; zst[0] separate
    # (only needed ~3us later, at step 0's h-update)
    d_pre = nc.dram_tensor(
        "pre", [128, CWE_COLS + SB], F16, kind="ExternalInput"
    )
    d_prez = nc.dram_tensor("prez", [128, SB], F16, kind="ExternalInput")
    d_cwm = nc.dram_tensor("cwm", [128, CWM_COLS], F16, kind="ExternalInput")
    # f32: b1|b2|dtb3(20)|gb1|gb2|rb1|rb2 -> 26 cols
    d_cf = nc.dram_tensor("cf", [128, 26], F32, kind="ExternalInput")
    d_out = nc.dram_tensor("out", [2, BC], F32, kind="ExternalOutput")

    with ExitStack() as ctx:
        tc = ctx.enter_context(tile.TileContext(nc))
        consts = ctx.enter_context(tc.tile_pool(name="consts", bufs=1))
        xzp = ctx.enter_context(tc.tile_pool(name="xzp", bufs=6))
        hp = ctx.enter_context(tc.tile_pool(name="hp", bufs=3))
        wk = ctx.enter_context(tc.tile_pool(name="wk", bufs=3))
        ppb = ctx.enter_context(tc.tile_pool(name="ppb", bufs=4, space="PSUM"))
        pps = ctx.enter_context(tc.tile_pool(name="pps", bufs=4, space="PSUM"))

        # ACT table preload before any DMA: read the (not yet written)
        # cwm tile -- the table load runs at t~0 and the cwm DMA (issued
        # below, WAR-ordered after this read) is only needed by step 1.
        cwm = consts.tile([128, CWM_COLS], F16, name="cwm", tag="cwm")
        atp = wk.tile([128, 1], F16, name="atp", tag="atp")
        nc.scalar.activation(atp[:], cwm[:, 0:1], AF.Sigmoid, bias=0.0)

        # DMA priority order: biases, step-0 bundle (weights + x + noise),
        # then the bulk consts; step-1 inputs issue from GpSimd in parallel.
        cf = consts.tile([128, 26], F32, name="cf", tag="cf")
        nc.sync.dma_start(cf[:], d_cf[:, :])
        pre = consts.tile([128, CWE_COLS + SB], F16, name="pre", tag="pre")
        nc.sync.dma_start(pre[:], d_pre[:, :])
        prez = consts.tile([128, SB], F16, name="prez", tag="prez")
        nc.sync.dma_start(prez[:], d_prez[:, :])
        nc.sync.dma_start(cwm[:], d_cwm[:, :])
        xz_pre = [(None, None)]
        xkb1 = xzp.tile([128, 2, HB], F16, name="xkb", tag="xkb")
        nc.sync.dma_start(xkb1[:], d_xt[1])
        zkb1 = xzp.tile([128, 2, HB], F16, name="zkb", tag="zkb")
        nc.sync.dma_start(zkb1[:], d_zst[1])
        xz_pre.append((xkb1, zkb1))

        w1x = pre[:, 0:128]
        w2 = pre[:, 128:256]
        w1h = cwm[:, W1H_OF : W1H_OF + 128]
        gw1 = cwm[:, GW1_OF : GW1_OF + 128]
        gw2 = cwm[:, GW2_OF : GW2_OF + 128]
        idn = cwm[:, IDN_OF : IDN_OF + 128]
        rw1 = cwm[:, RW1_OF : RW1_OF + 32]
        rw2 = cwm[0:64, RW2_OF : RW2_OF + 2]  # rw2 stacked twice
        b1 = cf[:, 0:1]
        b2 = cf[:, 1:2]
        dtb3 = cf[:, 2:22]
        gb1 = cf[:, 22:23]
        gb2 = cf[:, 23:24]
        rb1 = cf[0:32, 24:25]
        rb2 = cf[0:2, 25:26]

        h_cur = [None, None]

        def step_pair(k):
            """One Euler-Maruyama step for both streams.  All instruction
            issue (= engine FIFO order) follows dependency-arrival order:
            stream 0's stage-n work precedes stream 1 work that depends on
            the later h of stream 1."""
            if k == 0:
                xk = [pre[:, CWE_COLS + s * HB : CWE_COLS + (s + 1) * HB]
                      for s in range(2)]
                zk = [prez[:, s * HB : (s + 1) * HB] for s in range(2)]
            else:
                if k == 1:
                    xkb, zkb = xz_pre[1]
                else:
                    xkb = xzp.tile([128, 2, HB], F16, name="xkb", tag="xkb")
                    nc.sync.dma_start(xkb[:], d_xt[k])
                    zkb = xzp.tile([128, 2, HB], F16, name="zkb", tag="zkb")
                    nc.sync.dma_start(zkb[:], d_zst[k])
                xk = [xkb[:, s, :] for s in range(2)]
                zk = [zkb[:, s, :] for s in range(2)]

            g1, sg, tt, z1, z2 = {}, {}, {}, {}, {}
            psg, pss, ps1, ps2, ps3 = {}, {}, {}, {}, {}
            if k == 0:
                w3k = pre[:, 256:320]
            else:
                w3k = cwm[:, W3S_OF + H * (k - 1) : W3S_OF + H * k]

            def diff1_mm(s):
                psg[s] = pps.tile([128, HB], F32, name=f"psg{s}", tag="pps")
                nc.tensor.matmul(
                    psg[s][:, :], gw1[:, :], h_cur[s][:, :], start=True, stop=True
                )

            def drift1_mm(s):
                ps1[s] = [
                    ppb.tile([128, HB], F32, name=f"ps1{s}{j}", tag="ppb")
                    for j in range(2)
                ]
                for j, tp in ((0, None), (1, (64, 0))):
                    lo, hi = 64 * j, 64 * (j + 1)
                    dst = ps1[s][j][:, :]
                    if k > 0:
                        nc.tensor.matmul(
                            dst, w1h[lo:hi, :], h_cur[s][lo:hi, :],
                            start=True, stop=False, tile_position=tp,
                        )
                    nc.tensor.matmul(
                        dst, w1x[lo:hi, :], xk[s][lo:hi, :],
                        start=(k == 0), stop=True, tile_position=tp,
                    )

            def g1_op(s):
                # s0 on ACT; s1 alternates ACT/DVE by step parity to
                # balance the two engines' per-step load
                g1[s] = wk.tile([128, HB], F16, name=f"g1{s}", tag=f"g1{s}")
                if s == 0 or k % 2 == 0:
                    nc.scalar.activation(g1[s][:], psg[s][:], AF.Relu, bias=gb1[:])
                else:
                    nc.vector.tensor_scalar(
                        g1[s][:], psg[s][:], gb1[:], 0.0, ALU.add, ALU.max
                    )

            def diff2_mm(s):
                pss[s] = pps.tile([128, HB], F32, name=f"pss{s}", tag="pps")
                nc.tensor.matmul(
                    pss[s][:, :], gw2[:, :], g1[s][:, :], start=True, stop=True
                )

            def sig_tt(s):
                sg[s] = wk.tile([128, HB], F16, name=f"sg{s}", tag=f"sg{s}")
                nc.scalar.activation(sg[s][:], pss[s][:], AF.Sigmoid, bias=gb2[:])
                tt[s] = wk.tile([128, HB], F16, name=f"tt{s}", tag=f"tt{s}")
                nc.gpsimd.tensor_mul(tt[s][:], sg[s][:], zk[s][:])

            def relu1(s):
                z1[s] = wk.tile([128, SB], F16, name=f"z1{s}", tag=f"z1{s}")
                nc.scalar.activation(
                    z1[s][:, 0:HB], ps1[s][0][:], AF.Relu, bias=b1[:]
                )
                nc.vector.tensor_scalar(
                    z1[s][:, HB:], ps1[s][1][:], b1[:], 0.0, ALU.add, ALU.max
                )

            def drift2_mm(s):
                ps2[s] = [
                    ppb.tile([128, HB], F32, name=f"ps2{s}{j}", tag="ppb")
                    for j in range(2)
                ]
                for j in range(2):
                    nc.tensor.matmul(
                        ps2[s][j][:, :], w2[:, :], z1[s][:, j * HB : (j + 1) * HB],
                        start=True, stop=True,
                    )

            def relu2(s):
                z2[s] = wk.tile([128, SB], F16, name=f"z2{s}", tag=f"z2{s}")
                nc.scalar.activation(
                    z2[s][:, 0:HB], ps2[s][0][:], AF.Relu, bias=b2[:]
                )
                nc.vector.tensor_scalar(
                    z2[s][:, HB:], ps2[s][1][:], b2[:], 0.0, ALU.add, ALU.max
                )

            def out_mm(s):
                ps3[s] = pps.tile([128, HB], F32, name=f"ps3{s}", tag="pps")
                if k > 0:
                    nc.tensor.matmul(
                        ps3[s][:, :], idn[:, :], h_cur[s][:, :],
                        start=True, stop=False, skip_group_check=True,
                    )
                nc.tensor.matmul(
                    ps3[s][0:64, :], w3k, z2[s][:, 0:HB],
                    start=(k == 0), stop=False, skip_group_check=True,
                )
                nc.tensor.matmul(
                    ps3[s][64:128, :], w3k, z2[s][:, HB:],
                    start=(k == 0), stop=True, tile_position=(0, 64),
                    skip_group_check=True,
                )

            def h_upd(s):
                h_new = hp.tile([128, HB], F16, name=f"h{s}", tag=f"h{s}")
                nc.vector.scalar_tensor_tensor(
                    h_new[:], ps3[s][:], dtb3[:, k : k + 1], tt[s][:],
                    ALU.add, ALU.add,
                )
                h_cur[s] = h_new

            if k == 0:
                for s in range(2):
                    tt[s] = zk[s]
                    drift1_mm(s)
                for s in range(2):
                    relu1(s)
                    drift2_mm(s)
                for s in range(2):
                    relu2(s)
                    out_mm(s)
                for s in range(2):
                    h_upd(s)
                return

            # issue order = dependency-readiness order per engine FIFO
            diff1_mm(0)
            drift1_mm(0)
            g1_op(0)
            diff1_mm(1)
            drift1_mm(1)
            relu1(0)
            diff2_mm(0)
            sig_tt(0)
            g1_op(1)
            drift2_mm(0)
            relu1(1)
            diff2_mm(1)
            relu2(0)
            sig_tt(1)
            out_mm(0)
            drift2_mm(1)
            relu2(1)
            h_upd(0)
            out_mm(1)
            h_upd(1)

        for k in range(STEPS):
            step_pair(k)

        # ---- readout: out = relu(h @ rW1 + rb1) @ rW2 + rb2 ----
        # per stream: both 512-row halves land col-tiled in ONE psum tile
        # (partitions 0:32 half 0, 32:64 half 1) -> one relu per stream
        osb = wk.tile([2, BC], F32, name="osb", tag="osb")

        def ro(s):
            psr = pps.tile([128, HB], F32, name="psr", tag="pps")
            for j, tp in ((0, None), (1, (64, 32))):
                lo, hi = 64 * j, 64 * (j + 1)
                nc.tensor.matmul(
                    psr[32 * j : 32 * (j + 1), :], rw1[lo:hi, :],
                    h_cur[s][lo:hi, :],
                    start=True, stop=True, tile_position=tp,
                )
            r1 = wk.tile([64, HB], F16, name=f"r1{s}", tag=f"r1{s}")
            rb1d = cf[0:64, 24:25]
            if s == 0:
                nc.scalar.activation(r1[:, :], psr[0:64, :], AF.Relu, bias=rb1d)
            else:
                nc.vector.tensor_scalar(
                    r1[:, :], psr[0:64, :], rb1d, 0.0, ALU.add, ALU.max
                )
            for j in range(2):
                sl = slice(s * SB + j * HB, s * SB + (j + 1) * HB)
                pso = pps.tile([128, HB], F32, name="pso", tag="pps")
                nc.tensor.matmul(
                    pso[0:2, :], rw2[32 * j : 32 * (j + 1), :],
                    r1[32 * j : 32 * (j + 1), :],
                    start=True, stop=True, tile_position=(32 * j, 0),
                )
                if (s + j) % 2 == 0:
                    nc.scalar.activation(
                        osb[:, sl], pso[0:2, :], AF.Identity, bias=rb2[:]
                    )
                else:
                    nc.vector.tensor_scalar(
                        osb[:, sl], pso[0:2, :], rb2[:], 0.0, ALU.add, ALU.add
                    )
                nc.sync.dma_start(d_out[:, sl], osb[:, sl])

        ro(0)
        ro(1)

    nc.compile()
    _CACHE["nc"] = nc
    return nc


def _dup(a, dt=NPF16):
    return np.ascontiguousarray(np.concatenate([a, a], axis=0).astype(dt))


def _blkdiag(a, dt=NPF16):
    n, m = a.shape
    out = np.zeros((2 * n, 2 * m), np.float32)
    out[:n, :m] = a
    out[n:, m:] = a
    return np.ascontiguousarray(out.astype(dt))


def _prep_in_maps(inputs):
    xp = np.asarray(inputs["x_path"], dtype=np.float32)
    t_span = np.asarray(inputs["t_span"], dtype=np.float32)
    dw = np.asarray(inputs["dW"], dtype=np.float32)

    Tm1 = np.int32(xp.shape[1] - 1)
    t_max = t_span[-1]
    idx = np.clip(
        (t_span[:-1] / t_max * np.float32(Tm1)).astype(np.int32), 0, Tm1
    )
    dts = (t_span[1:] - t_span[:-1]).astype(np.float32)
    sq = np.sqrt(dts).astype(np.float32)

    gscale = np.asarray(inputs["gscale"], dtype=np.float32)
    w1 = np.asarray(inputs["dW1"], dtype=np.float32)
    w2 = np.asarray(inputs["dW2"], dtype=np.float32)
    w3 = np.asarray(inputs["dW3"], dtype=np.float32)
    db1 = np.asarray(inputs["db1"], dtype=np.float32)
    db2 = np.asarray(inputs["db2"], dtype=np.float32)
    db3 = np.asarray(inputs["db3"], dtype=np.float32)
    gw1 = np.asarray(inputs["gW1"], dtype=np.float32)
    gw2 = np.asarray(inputs["gW2"], dtype=np.float32)
    gb1 = np.asarray(inputs["gb1"], dtype=np.float32)
    gb2 = np.asarray(inputs["gb2"], dtype=np.float32)
    rw1 = np.asarray(inputs["rW1"], dtype=np.float32)
    rb1 = np.asarray(inputs["rb1"], dtype=np.float32)
    rw2 = np.asarray(inputs["rW2"], dtype=np.float32)
    rb2 = np.asarray(inputs["rb2"], dtype=np.float32)

    w3s = w3[None, :, :] * dts[:, None, None]  # [STEPS, DW, H]

    def pad128(a):
        out = np.zeros((128, a.shape[1]), a.dtype)
        out[: a.shape[0]] = a
        return out

    cwe_pack = np.concatenate(
        [
            _dup(w1[H:]),  # w1x
            w2.astype(NPF16),  # w2
            w3s[0].astype(NPF16),  # w3s step 0
        ],
        axis=1,
    )
    w3s_flat = w3s[1:].transpose(1, 0, 2).reshape(DW, (STEPS - 1) * H)
    cwm_pack = np.concatenate(
        [
            _dup(w1[:H]),  # w1h
            w3s_flat.astype(NPF16),  # w3s steps 1..19
            _blkdiag(gw1),  # gw1
            _blkdiag(gw2),  # gw2
            np.eye(DW, dtype=NPF16),  # ident
            _dup(rw1),  # rw1
            pad128(_dup(rw2)),  # rw2 stacked twice (row-tiled readout)
        ],
        axis=1,
    )
    cf_pack = np.concatenate(
        [
            db1.reshape(DW, 1),
            db2.reshape(DW, 1),
            _dup((dts[:, None] * db3[None, :]).T, np.float32),
            _dup(gb1.reshape(H, 1), np.float32),
            _dup(gb2.reshape(H, 1), np.float32),
            pad128(_dup(rb1.reshape(32, 1), np.float32)),
            pad128(rb2.reshape(2, 1)),
        ],
        axis=1,
    ).astype(np.float32)

    common = {
        "cwm": np.ascontiguousarray(cwm_pack),
        "cf": np.ascontiguousarray(cf_pack),
    }

    xg = xp[:, idx, :]  # [B, STEPS, F]
    # noise pre-scale; step 0's constant sigmoid folded in (h_0 = 0)
    g1c = np.maximum(gb1, 0.0)
    sg0 = 1.0 / (1.0 + np.exp(-(g1c @ gw2 + gb2)))
    zsc = gscale[None, :] * sq[:, None]  # [STEPS, F]
    zsc[0] *= sg0

    in_maps = []
    for c in range(NCORES):
        rows = slice(c * BC, (c + 1) * BC)
        # (stream, half, b', k, f) -> (k, stream, half, f, b')
        xt = np.ascontiguousarray(
            xg[rows]
            .reshape(2, 2, HB, STEPS, FX)
            .transpose(3, 1, 4, 0, 2)
            .reshape(STEPS, 128, 2, HB)
            .astype(NPF16)
        )
        zc = dw[:, rows, :] * zsc[:, None, :]  # [STEPS, BC, H]
        zst = np.ascontiguousarray(
            zc.reshape(STEPS, 2, 2, HB, H)
            .transpose(0, 2, 4, 1, 3)
            .reshape(STEPS, 128, 2, HB)
            .astype(NPF16)
        )
        m = dict(common)
        m["xt"] = xt
        m["zst"] = zst
        m["pre"] = np.ascontiguousarray(
            np.concatenate([cwe_pack, xt[0].reshape(128, SB)], axis=1)
        )
        m["prez"] = np.ascontiguousarray(zst[0].reshape(128, SB))
        in_maps.append(m)
    return in_maps


def kernel(**inputs):
    nc = _build()
    in_maps = _prep_in_maps(inputs)
    run_kwargs = dict(_CACHE.get("run_kwargs", {}))
    res = run_bass_kernel_spmd(nc, in_maps, list(range(NCORES)), **run_kwargs)
    _CACHE["last_results"] = res
    mu = np.concatenate([res.results[c]["out"][0] for c in range(NCORES)])
    ls = np.concatenate([res.results[c]["out"][1] for c in range(NCORES)])
    return mu, ls


# revision 46
# speedup vs baseline: 1.0004x; 1.0004x over previous
"""Trainium2 Bass kernel for the NSDE model (Euler-Maruyama scan + MLPs).

Strategy:
  - Data-parallel over batch: 16384 rows -> 8 cores x 2048 rows.
  - Only the 20 time slices of x_path that the scan actually reads are
    shipped to the device (indices computed on host from t_span).
  - Feature-major layout on chip: activations are [feature, batch] so every
    matmul uses the weight matrix directly as lhsT (out = W^T @ actT) and
    biases are per-partition scalars.
  - The per-core batch (2048) is processed as TWO independent interleaved
    streams of 1024 rows; 64-feature tensors are "packed": partitions 0-63
    hold features of a stream's first 512 rows, partitions 64-127 the
    second 512.
  - Everything runs in fp16 (weights, x, noise, and the h carry): fp16
    matmuls stream 1 col/cycle at full clock (f32r runs at half clock) and
    fp16's 11-bit mantissa keeps the 20-step carry accurate (~1e-3 final
    rel err, validated against the reference in float simulation).
  - The h carry is added into the drift-out PSUM exactly via an fp16
    identity matmul (1.0 * h is exact), so h' = (psum3 + dt*db3) + sig*zs
    is a single fused scalar_tensor_tensor per stream.
  - Engine queues are strict FIFO, so per-step instructions are issued in
    dependency-arrival order: stream 1 trails stream 0, and every engine's
    queue lists all stream-0 work for a stage before stream-1 work that
    becomes ready later (avoids head-of-line blocking).
  - Elementwise split per stream: ACT does relu/sigmoid halves, DVE does
    the other halves + the fused h-update, GpSimd does the noise multiply
    (the only SBUF-only op -- GpSimd has no PSUM port).
  - dt and sqrt(dt) folds on host: dW3*dt per step, db3*dt, noise
    pre-scaled zs = dW * gscale * sqrt(dt). Step 0 is specialized (h=0):
    no diffusion net, no w1h matmuls, and sigmoid(const) is folded into
    zs[0] on the host.
  - Prologue: a tiny garbage tile plus the step-0 weight slice (w1x, w2,
    dt0*w3) are DMA'd before the bulk consts so the first drift matmuls
    and the PE warm-up start as early as possible.
"""

import os
from contextlib import ExitStack

import numpy as np

import concourse.bass as bass
import concourse.mybir as mybir
import concourse.tile as tile
from concourse import bacc
from concourse.bass_utils import run_bass_kernel_spmd

F32 = mybir.dt.float32
F16 = mybir.dt.float16
AF = mybir.ActivationFunctionType
ALU = mybir.AluOpType

NPF16 = np.float16

STEPS = 20
NCORES = 8
B = 16384
BC = B // NCORES  # per-core batch: 2048
SB = BC // 2  # per-stream batch: 1024
HB = SB // 2  # packed free size per stream: 512
H = 64
FX = 64
DW = 128

# cw_early columns: w1x | w2 | w3s[0]
CWE_COLS = 128 + 128 + 64  # 320
# cw_main columns: w1h | w3s[1:20] | gw1 | gw2 | idn | rw1 | rw2
W1H_OF = 0
W3S_OF = 128  # steps 1..19 -> 128 + 64*(k-1)
GW1_OF = W3S_OF + (STEPS - 1) * H  # 1344
GW2_OF = GW1_OF + 128  # 1472
IDN_OF = GW2_OF + 128  # 1600
RW1_OF = IDN_OF + 128  # 1728
RW2_OF = RW1_OF + 32  # 1760
CWM_COLS = RW2_OF + 2  # 1762

_CACHE = {}


def _build():
    if "nc" in _CACHE:
        return _CACHE["nc"]

    nc = bacc.Bacc("TRN2", target_bir_lowering=False, debug=False)

    d_xt = nc.dram_tensor("xt", [STEPS, 128, 2, HB], F16, kind="ExternalInput")
    d_zst = nc.dram_tensor("zst", [STEPS, 128, 2, HB], F16, kind="ExternalInput")
    # step-0 bundle: cwe weights | xt[0] | zst[0] in one DMA
    d_pre = nc.dram_tensor(
        "pre", [128, CWE_COLS + 2 * SB], F16, kind="ExternalInput"
    )
    d_cwm = nc.dram_tensor("cwm", [128, CWM_COLS], F16, kind="ExternalInput")
    # f32: b1|b2|dtb3(20)|gb1|gb2|rb1|rb2 -> 26 cols
    d_cf = nc.dram_tensor("cf", [128, 26], F32, kind="ExternalInput")
    d_out = nc.dram_tensor("out", [2, BC], F32, kind="ExternalOutput")

    with ExitStack() as ctx:
        tc = ctx.enter_context(tile.TileContext(nc))
        consts = ctx.enter_context(tc.tile_pool(name="consts", bufs=1))
        xzp = ctx.enter_context(tc.tile_pool(name="xzp", bufs=6))
        hp = ctx.enter_context(tc.tile_pool(name="hp", bufs=3))
        wk = ctx.enter_context(tc.tile_pool(name="wk", bufs=3))
        ppb = ctx.enter_context(tc.tile_pool(name="ppb", bufs=4, space="PSUM"))
        pps = ctx.enter_context(tc.tile_pool(name="pps", bufs=4, space="PSUM"))

        # ACT table preload before any DMA: read the (not yet written)
        # cwm tile -- the table load runs at t~0 and the cwm DMA (issued
        # below, WAR-ordered after this read) is only needed by step 1.
        cwm = consts.tile([128, CWM_COLS], F16, name="cwm", tag="cwm")
        atp = wk.tile([128, 1], F16, name="atp", tag="atp")
        nc.scalar.activation(atp[:], cwm[:, 0:1], AF.Sigmoid, bias=0.0)

        # DMA priority order: biases, step-0 bundle (weights + x + noise),
        # then the bulk consts; step-1 inputs issue from GpSimd in parallel.
        cf = consts.tile([128, 26], F32, name="cf", tag="cf")
        nc.sync.dma_start(cf[:], d_cf[:, :])
        pre = consts.tile([128, CWE_COLS + 2 * SB], F16, name="pre", tag="pre")
        nc.sync.dma_start(pre[:], d_pre[:, :])
        nc.sync.dma_start(cwm[:], d_cwm[:, :])
        xz_pre = [(None, None)]
        xkb1 = xzp.tile([128, 2, HB], F16, name="xkb", tag="xkb")
        nc.sync.dma_start(xkb1[:], d_xt[1])
        zkb1 = xzp.tile([128, 2, HB], F16, name="zkb", tag="zkb")
        nc.sync.dma_start(zkb1[:], d_zst[1])
        xz_pre.append((xkb1, zkb1))

        w1x = pre[:, 0:128]
        w2 = pre[:, 128:256]
        w1h = cwm[:, W1H_OF : W1H_OF + 128]
        gw1 = cwm[:, GW1_OF : GW1_OF + 128]
        gw2 = cwm[:, GW2_OF : GW2_OF + 128]
        idn = cwm[:, IDN_OF : IDN_OF + 128]
        rw1 = cwm[:, RW1_OF : RW1_OF + 32]
        rw2 = cwm[0:64, RW2_OF : RW2_OF + 2]  # rw2 stacked twice
        b1 = cf[:, 0:1]
        b2 = cf[:, 1:2]
        dtb3 = cf[:, 2:22]
        gb1 = cf[:, 22:23]
        gb2 = cf[:, 23:24]
        rb1 = cf[0:32, 24:25]
        rb2 = cf[0:2, 25:26]

        h_cur = [None, None]

        def step_pair(k):
            """One Euler-Maruyama step for both streams.  All instruction
            issue (= engine FIFO order) follows dependency-arrival order:
            stream 0's stage-n work precedes stream 1 work that depends on
            the later h of stream 1."""
            if k == 0:
                xk = [pre[:, CWE_COLS + s * HB : CWE_COLS + (s + 1) * HB]
                      for s in range(2)]
                zk = [pre[:, CWE_COLS + SB + s * HB : CWE_COLS + SB + (s + 1) * HB]
                      for s in range(2)]
            else:
                if k == 1:
                    xkb, zkb = xz_pre[1]
                else:
                    xkb = xzp.tile([128, 2, HB], F16, name="xkb", tag="xkb")
                    nc.sync.dma_start(xkb[:], d_xt[k])
                    zkb = xzp.tile([128, 2, HB], F16, name="zkb", tag="zkb")
                    nc.sync.dma_start(zkb[:], d_zst[k])
                xk = [xkb[:, s, :] for s in range(2)]
                zk = [zkb[:, s, :] for s in range(2)]

            g1, sg, tt, z1, z2 = {}, {}, {}, {}, {}
            psg, pss, ps1, ps2, ps3 = {}, {}, {}, {}, {}
            if k == 0:
                w3k = pre[:, 256:320]
            else:
                w3k = cwm[:, W3S_OF + H * (k - 1) : W3S_OF + H * k]

            def diff1_mm(s):
                psg[s] = pps.tile([128, HB], F32, name=f"psg{s}", tag="pps")
                nc.tensor.matmul(
                    psg[s][:, :], gw1[:, :], h_cur[s][:, :], start=True, stop=True
                )

            def drift1_mm(s):
                ps1[s] = [
                    ppb.tile([128, HB], F32, name=f"ps1{s}{j}", tag="ppb")
                    for j in range(2)
                ]
                for j, tp in ((0, None), (1, (64, 0))):
                    lo, hi = 64 * j, 64 * (j + 1)
                    dst = ps1[s][j][:, :]
                    if k > 0:
                        nc.tensor.matmul(
                            dst, w1h[lo:hi, :], h_cur[s][lo:hi, :],
                            start=True, stop=False, tile_position=tp,
                        )
                    nc.tensor.matmul(
                        dst, w1x[lo:hi, :], xk[s][lo:hi, :],
                        start=(k == 0), stop=True, tile_position=tp,
                    )

            def g1_op(s):
                # s0 on ACT; s1 alternates ACT/DVE by step parity to
                # balance the two engines' per-step load
                g1[s] = wk.tile([128, HB], F16, name=f"g1{s}", tag=f"g1{s}")
                if s == 0 or k % 2 == 0:
                    nc.scalar.activation(g1[s][:], psg[s][:], AF.Relu, bias=gb1[:])
                else:
                    nc.vector.tensor_scalar(
                        g1[s][:], psg[s][:], gb1[:], 0.0, ALU.add, ALU.max
                    )

            def diff2_mm(s):
                pss[s] = pps.tile([128, HB], F32, name=f"pss{s}", tag="pps")
                nc.tensor.matmul(
                    pss[s][:, :], gw2[:, :], g1[s][:, :], start=True, stop=True
                )

            def sig_tt(s):
                sg[s] = wk.tile([128, HB], F16, name=f"sg{s}", tag=f"sg{s}")
                nc.scalar.activation(sg[s][:], pss[s][:], AF.Sigmoid, bias=gb2[:])
                tt[s] = wk.tile([128, HB], F16, name=f"tt{s}", tag=f"tt{s}")
                nc.gpsimd.tensor_mul(tt[s][:], sg[s][:], zk[s][:])

            def relu1(s):
                z1[s] = wk.tile([128, SB], F16, name=f"z1{s}", tag=f"z1{s}")
                nc.scalar.activation(
                    z1[s][:, 0:HB], ps1[s][0][:], AF.Relu, bias=b1[:]
                )
                nc.vector.tensor_scalar(
                    z1[s][:, HB:], ps1[s][1][:], b1[:], 0.0, ALU.add, ALU.max
                )

            def drift2_mm(s):
                ps2[s] = [
                    ppb.tile([128, HB], F32, name=f"ps2{s}{j}", tag="ppb")
                    for j in range(2)
                ]
                for j in range(2):
                    nc.tensor.matmul(
                        ps2[s][j][:, :], w2[:, :], z1[s][:, j * HB : (j + 1) * HB],
                        start=True, stop=True,
                    )

            def relu2(s):
                z2[s] = wk.tile([128, SB], F16, name=f"z2{s}", tag=f"z2{s}")
                nc.scalar.activation(
                    z2[s][:, 0:HB], ps2[s][0][:], AF.Relu, bias=b2[:]
                )
                nc.vector.tensor_scalar(
                    z2[s][:, HB:], ps2[s][1][:], b2[:], 0.0, ALU.add, ALU.max
                )

            def out_mm(s):
                ps3[s] = pps.tile([128, HB], F32, name=f"ps3{s}", tag="pps")
                if k > 0:
                    nc.tensor.matmul(
                        ps3[s][:, :], idn[:, :], h_cur[s][:, :],
                        start=True, stop=False, skip_group_check=True,
                    )
                nc.tensor.matmul(
                    ps3[s][0:64, :], w3k, z2[s][:, 0:HB],
                    start=(k == 0), stop=False, skip_group_check=True,
                )
                nc.tensor.matmul(
                    ps3[s][64:128, :], w3k, z2[s][:, HB:],
                    start=(k == 0), stop=True, tile_position=(0, 64),
                    skip_group_check=True,
                )

            def h_upd(s):
                h_new = hp.tile([128, HB], F16, name=f"h{s}", tag=f"h{s}")
                nc.vector.scalar_tensor_tensor(
                    h_new[:], ps3[s][:], dtb3[:, k : k + 1], tt[s][:],
                    ALU.add, ALU.add,
                )
                h_cur[s] = h_new

            if k == 0:
                for s in range(2):
                    tt[s] = zk[s]
                    drift1_mm(s)
                for s in range(2):
                    relu1(s)
                    drift2_mm(s)
                for s in range(2):
                    relu2(s)
                    out_mm(s)
                for s in range(2):
                    h_upd(s)
                return

            # issue order = dependency-readiness order per engine FIFO
            diff1_mm(0)
            drift1_mm(0)
            g1_op(0)
            diff1_mm(1)
            drift1_mm(1)
            relu1(0)
            diff2_mm(0)
            sig_tt(0)
            g1_op(1)
            drift2_mm(0)
            relu1(1)
            diff2_mm(1)
            relu2(0)
            sig_tt(1)
            out_mm(0)
            drift2_mm(1)
            relu2(1)
            h_upd(0)
            out_mm(1)
            h_upd(1)

        for k in range(STEPS):
            step_pair(k)

        # ---- readout: out = relu(h @ rW1 + rb1) @ rW2 + rb2 ----
        # per stream: both 512-row halves land col-tiled in ONE psum tile
        # (partitions 0:32 half 0, 32:64 half 1) -> one relu per stream
        osb = wk.tile([2, BC], F32, name="osb", tag="osb")

        def ro(s):
            psr = pps.tile([128, HB], F32, name="psr", tag="pps")
            for j, tp in ((0, None), (1, (64, 32))):
                lo, hi = 64 * j, 64 * (j + 1)
                nc.tensor.matmul(
                    psr[32 * j : 32 * (j + 1), :], rw1[lo:hi, :],
                    h_cur[s][lo:hi, :],
                    start=True, stop=True, tile_position=tp,
                )
            r1 = wk.tile([64, HB], F16, name=f"r1{s}", tag=f"r1{s}")
            rb1d = cf[0:64, 24:25]
            if s == 0:
                nc.scalar.activation(r1[:, :], psr[0:64, :], AF.Relu, bias=rb1d)
            else:
                nc.vector.tensor_scalar(
                    r1[:, :], psr[0:64, :], rb1d, 0.0, ALU.add, ALU.max
                )
            for j in range(2):
                sl = slice(s * SB + j * HB, s * SB + (j + 1) * HB)
                pso = pps.tile([128, HB], F32, name="pso", tag="pps")
                nc.tensor.matmul(
                    pso[0:2, :], rw2[32 * j : 32 * (j + 1), :],
                    r1[32 * j : 32 * (j + 1), :],
                    start=True, stop=True, tile_position=(32 * j, 0),
                )
                if (s + j) % 2 == 0:
                    nc.scalar.activation(
                        osb[:, sl], pso[0:2, :], AF.Identity, bias=rb2[:]
                    )
                else:
                    nc.vector.tensor_scalar(
                        osb[:, sl], pso[0:2, :], rb2[:], 0.0, ALU.add, ALU.add
                    )
                nc.sync.dma_start(d_out[:, sl], osb[:, sl])

        ro(0)
        ro(1)

    nc.compile()
    _CACHE["nc"] = nc
    return nc


def _dup(a, dt=NPF16):
    return np.ascontiguousarray(np.concatenate([a, a], axis=0).astype(dt))


def _blkdiag(a, dt=NPF16):
    n, m = a.shape
    out = np.zeros((2 * n, 2 * m), np.float32)
    out[:n, :m] = a
    out[n:, m:] = a
    return np.ascontiguousarray(out.astype(dt))


def _prep_in_maps(inputs):
    xp = np.asarray(inputs["x_path"], dtype=np.float32)
    t_span = np.asarray(inputs["t_span"], dtype=np.float32)
    dw = np.asarray(inputs["dW"], dtype=np.float32)

    Tm1 = np.int32(xp.shape[1] - 1)
    t_max = t_span[-1]
    idx = np.clip(
        (t_span[:-1] / t_max * np.float32(Tm1)).astype(np.int32), 0, Tm1
    )
    dts = (t_span[1:] - t_span[:-1]).astype(np.float32)
    sq = np.sqrt(dts).astype(np.float32)

    gscale = np.asarray(inputs["gscale"], dtype=np.float32)
    w1 = np.asarray(inputs["dW1"], dtype=np.float32)
    w2 = np.asarray(inputs["dW2"], dtype=np.float32)
    w3 = np.asarray(inputs["dW3"], dtype=np.float32)
    db1 = np.asarray(inputs["db1"], dtype=np.float32)
    db2 = np.asarray(inputs["db2"], dtype=np.float32)
    db3 = np.asarray(inputs["db3"], dtype=np.float32)
    gw1 = np.asarray(inputs["gW1"], dtype=np.float32)
    gw2 = np.asarray(inputs["gW2"], dtype=np.float32)
    gb1 = np.asarray(inputs["gb1"], dtype=np.float32)
    gb2 = np.asarray(inputs["gb2"], dtype=np.float32)
    rw1 = np.asarray(inputs["rW1"], dtype=np.float32)
    rb1 = np.asarray(inputs["rb1"], dtype=np.float32)
    rw2 = np.asarray(inputs["rW2"], dtype=np.float32)
    rb2 = np.asarray(inputs["rb2"], dtype=np.float32)

    w3s = w3[None, :, :] * dts[:, None, None]  # [STEPS, DW, H]

    def pad128(a):
        out = np.zeros((128, a.shape[1]), a.dtype)
        out[: a.shape[0]] = a
        return out

    cwe_pack = np.concatenate(
        [
            _dup(w1[H:]),  # w1x
            w2.astype(NPF16),  # w2
            w3s[0].astype(NPF16),  # w3s step 0
        ],
        axis=1,
    )
    w3s_flat = w3s[1:].transpose(1, 0, 2).reshape(DW, (STEPS - 1) * H)
    cwm_pack = np.concatenate(
        [
            _dup(w1[:H]),  # w1h
            w3s_flat.astype(NPF16),  # w3s steps 1..19
            _blkdiag(gw1),  # gw1
            _blkdiag(gw2),  # gw2
            np.eye(DW, dtype=NPF16),  # ident
            _dup(rw1),  # rw1
            pad128(_dup(rw2)),  # rw2 stacked twice (row-tiled readout)
        ],
        axis=1,
    )
    cf_pack = np.concatenate(
        [
            db1.reshape(DW, 1),
            db2.reshape(DW, 1),
            _dup((dts[:, None] * db3[None, :]).T, np.float32),
            _dup(gb1.reshape(H, 1), np.float32),
            _dup(gb2.reshape(H, 1), np.float32),
            pad128(_dup(rb1.reshape(32, 1), np.float32)),
            pad128(rb2.reshape(2, 1)),
        ],
        axis=1,
    ).astype(np.float32)

    common = {
        "cwm": np.ascontiguousarray(cwm_pack),
        "cf": np.ascontiguousarray(cf_pack),
    }

    xg = xp[:, idx, :]  # [B, STEPS, F]
    # noise pre-scale; step 0's constant sigmoid folded in (h_0 = 0)
    g1c = np.maximum(gb1, 0.0)
    sg0 = 1.0 / (1.0 + np.exp(-(g1c @ gw2 + gb2)))
    zsc = gscale[None, :] * sq[:, None]  # [STEPS, F]
    zsc[0] *= sg0

    in_maps = []
    for c in range(NCORES):
        rows = slice(c * BC, (c + 1) * BC)
        # (stream, half, b', k, f) -> (k, stream, half, f, b')
        xt = np.ascontiguousarray(
            xg[rows]
            .reshape(2, 2, HB, STEPS, FX)
            .transpose(3, 1, 4, 0, 2)
            .reshape(STEPS, 128, 2, HB)
            .astype(NPF16)
        )
        zc = dw[:, rows, :] * zsc[:, None, :]  # [STEPS, BC, H]
        zst = np.ascontiguousarray(
            zc.reshape(STEPS, 2, 2, HB, H)
            .transpose(0, 2, 4, 1, 3)
            .reshape(STEPS, 128, 2, HB)
            .astype(NPF16)
        )
        m = dict(common)
        m["xt"] = xt
        m["zst"] = zst
        m["pre"] = np.ascontiguousarray(
            np.concatenate(
                [cwe_pack, xt[0].reshape(128, SB), zst[0].reshape(128, SB)],
                axis=1,
            )
        )
        in_maps.append(m)
    return in_maps


def kernel(**inputs):
    nc = _build()
    in_maps = _prep_in_maps(inputs)
    run_kwargs = dict(_CACHE.get("run_kwargs", {}))
    res = run_bass_kernel_spmd(nc, in_maps, list(range(NCORES)), **run_kwargs)
    _CACHE["last_results"] = res
    mu = np.concatenate([res.results[c]["out"][0] for c in range(NCORES)])
    ls = np.concatenate([res.results[c]["out"][1] for c in range(NCORES)])
    return mu, ls


# revision 47
# speedup vs baseline: 1.0069x; 1.0065x over previous
"""Trainium2 Bass kernel for the NSDE model (Euler-Maruyama scan + MLPs).

Strategy:
  - Data-parallel over batch: 16384 rows -> 8 cores x 2048 rows.
  - Only the 20 time slices of x_path that the scan actually reads are
    shipped to the device (indices computed on host from t_span).
  - Feature-major layout on chip: activations are [feature, batch] so every
    matmul uses the weight matrix directly as lhsT (out = W^T @ actT) and
    biases are per-partition scalars.
  - The per-core batch (2048) is processed as TWO independent interleaved
    streams of 1024 rows; 64-feature tensors are "packed": partitions 0-63
    hold features of a stream's first 512 rows, partitions 64-127 the
    second 512.
  - Everything runs in fp16 (weights, x, noise, and the h carry): fp16
    matmuls stream 1 col/cycle at full clock (f32r runs at half clock) and
    fp16's 11-bit mantissa keeps the 20-step carry accurate (~1e-3 final
    rel err, validated against the reference in float simulation).
  - The h carry is added into the drift-out PSUM exactly via an fp16
    identity matmul (1.0 * h is exact), so h' = (psum3 + dt*db3) + sig*zs
    is a single fused scalar_tensor_tensor per stream.
  - Engine queues are strict FIFO, so per-step instructions are issued in
    dependency-arrival order: stream 1 trails stream 0, and every engine's
    queue lists all stream-0 work for a stage before stream-1 work that
    becomes ready later (avoids head-of-line blocking).
  - Elementwise split per stream: ACT does relu/sigmoid halves, DVE does
    the other halves + the fused h-update, GpSimd does the noise multiply
    (the only SBUF-only op -- GpSimd has no PSUM port).
  - dt and sqrt(dt) folds on host: dW3*dt per step, db3*dt, noise
    pre-scaled zs = dW * gscale * sqrt(dt). Step 0 is specialized (h=0):
    no diffusion net, no w1h matmuls, and sigmoid(const) is folded into
    zs[0] on the host.
  - Prologue: a tiny garbage tile plus the step-0 weight slice (w1x, w2,
    dt0*w3) are DMA'd before the bulk consts so the first drift matmuls
    and the PE warm-up start as early as possible.
"""

import os
from contextlib import ExitStack

import numpy as np

import concourse.bass as bass
import concourse.mybir as mybir
import concourse.tile as tile
from concourse import bacc
from concourse.bass_utils import run_bass_kernel_spmd

F32 = mybir.dt.float32
F16 = mybir.dt.float16
AF = mybir.ActivationFunctionType
ALU = mybir.AluOpType

NPF16 = np.float16

STEPS = 20
NCORES = 8
B = 16384
BC = B // NCORES  # per-core batch: 2048
SB = BC // 2  # per-stream batch: 1024
HB = SB // 2  # packed free size per stream: 512
H = 64
FX = 64
DW = 128

# cw_early columns: w1x | w2 | w3s[0]
CWE_COLS = 128 + 128 + 64  # 320
# cw_main columns: w1h | w3s[1:20] | gw1 | gw2 | idn | rw1 | rw2
W1H_OF = 0
W3S_OF = 128  # steps 1..19 -> 128 + 64*(k-1)
GW1_OF = W3S_OF + (STEPS - 1) * H  # 1344
GW2_OF = GW1_OF + 128  # 1472
IDN_OF = GW2_OF + 128  # 1600
RW1_OF = IDN_OF + 128  # 1728
RW2_OF = RW1_OF + 32  # 1760
CWM_COLS = RW2_OF + 2  # 1762

_CACHE = {}


def _build():
    if "nc" in _CACHE:
        return _CACHE["nc"]

    nc = bacc.Bacc("TRN2", target_bir_lowering=False, debug=False)

    d_xt = nc.dram_tensor("xt", [STEPS, 128, 2, HB], F16, kind="ExternalInput")
    d_zst = nc.dram_tensor("zst", [STEPS, 128, 2, HB], F16, kind="ExternalInput")
    # step-0 bundle: cwe weights | xt[0] | zst[0] in one DMA
    d_pre = nc.dram_tensor(
        "pre", [128, CWE_COLS + 2 * SB], F16, kind="ExternalInput"
    )
    d_cwm = nc.dram_tensor("cwm", [128, CWM_COLS], F16, kind="ExternalInput")
    # f32: b1|b2|dtb3(20)|gb1|gb2|rb1|rb2 -> 26 cols
    d_cf = nc.dram_tensor("cf", [128, 26], F32, kind="ExternalInput")
    d_out = nc.dram_tensor("out", [2, BC], F32, kind="ExternalOutput")

    with ExitStack() as ctx:
        tc = ctx.enter_context(tile.TileContext(nc))
        consts = ctx.enter_context(tc.tile_pool(name="consts", bufs=1))
        xzp = ctx.enter_context(tc.tile_pool(name="xzp", bufs=6))
        hp = ctx.enter_context(tc.tile_pool(name="hp", bufs=3))
        wk = ctx.enter_context(tc.tile_pool(name="wk", bufs=3))
        ppb = ctx.enter_context(tc.tile_pool(name="ppb", bufs=4, space="PSUM"))
        pps = ctx.enter_context(tc.tile_pool(name="pps", bufs=4, space="PSUM"))

        # ACT table preload before any DMA: read the (not yet written)
        # cwm tile -- the table load runs at t~0 and the cwm DMA (issued
        # below, WAR-ordered after this read) is only needed by step 1.
        cwm = consts.tile([128, CWM_COLS], F16, name="cwm", tag="cwm")
        atp = wk.tile([128, 1], F16, name="atp", tag="atp")
        nc.scalar.activation(atp[:], cwm[:, 0:1], AF.Sigmoid, bias=0.0)

        # DMA priority order: biases, step-0 bundle (weights + x + noise),
        # then the bulk consts; step-1 inputs issue from GpSimd in parallel.
        cf = consts.tile([128, 26], F32, name="cf", tag="cf")
        nc.sync.dma_start(cf[:], d_cf[:, :])
        pre = consts.tile([128, CWE_COLS + 2 * SB], F16, name="pre", tag="pre")
        nc.sync.dma_start(pre[:], d_pre[:, :])
        nc.sync.dma_start(cwm[:], d_cwm[:, :])
        xz_pre = [(None, None)]
        xkb1 = xzp.tile([128, 2, HB], F16, name="xkb", tag="xkb")
        nc.sync.dma_start(xkb1[:], d_xt[1])
        zkb1 = xzp.tile([128, 2, HB], F16, name="zkb", tag="zkb")
        nc.sync.dma_start(zkb1[:], d_zst[1])
        xz_pre.append((xkb1, zkb1))

        w1x = pre[:, 0:128]
        w2 = pre[:, 128:256]
        w1h = cwm[:, W1H_OF : W1H_OF + 128]
        gw1 = cwm[:, GW1_OF : GW1_OF + 128]
        gw2 = cwm[:, GW2_OF : GW2_OF + 128]
        idn = cwm[:, IDN_OF : IDN_OF + 128]
        rw1 = cwm[:, RW1_OF : RW1_OF + 32]
        rw2 = cwm[0:64, RW2_OF : RW2_OF + 2]  # rw2 stacked twice
        b1 = cf[:, 0:1]
        b2 = cf[:, 1:2]
        dtb3 = cf[:, 2:22]
        gb1 = cf[:, 22:23]
        gb2 = cf[:, 23:24]
        rb1 = cf[0:32, 24:25]
        rb2 = cf[0:2, 25:26]

        h_cur = [None, None]

        def step_pair(k):
            """One Euler-Maruyama step for both streams.  All instruction
            issue (= engine FIFO order) follows dependency-arrival order:
            stream 0's stage-n work precedes stream 1 work that depends on
            the later h of stream 1."""
            if k == 0:
                xk = [pre[:, CWE_COLS + s * HB : CWE_COLS + (s + 1) * HB]
                      for s in range(2)]
                zk = [pre[:, CWE_COLS + SB + s * HB : CWE_COLS + SB + (s + 1) * HB]
                      for s in range(2)]
            else:
                if k == 1:
                    xkb, zkb = xz_pre[1]
                else:
                    xkb = xzp.tile([128, 2, HB], F16, name="xkb", tag="xkb")
                    nc.sync.dma_start(xkb[:], d_xt[k])
                    zkb = xzp.tile([128, 2, HB], F16, name="zkb", tag="zkb")
                    nc.sync.dma_start(zkb[:], d_zst[k])
                xk = [xkb[:, s, :] for s in range(2)]
                zk = [zkb[:, s, :] for s in range(2)]

            g1, sg, tt, z1, z2 = {}, {}, {}, {}, {}
            psg, pss, ps1, ps2, ps3 = {}, {}, {}, {}, {}
            if k == 0:
                w3k = pre[:, 256:320]
            else:
                w3k = cwm[:, W3S_OF + H * (k - 1) : W3S_OF + H * k]

            def diff1_mm(s):
                psg[s] = pps.tile([128, HB], F32, name=f"psg{s}", tag="pps")
                nc.tensor.matmul(
                    psg[s][:, :], gw1[:, :], h_cur[s][:, :], start=True, stop=True
                )

            def drift1_mm(s):
                ps1[s] = [
                    ppb.tile([128, HB], F32, name=f"ps1{s}{j}", tag="ppb")
                    for j in range(2)
                ]
                for j, tp in ((0, None), (1, (64, 0))):
                    lo, hi = 64 * j, 64 * (j + 1)
                    dst = ps1[s][j][:, :]
                    if k > 0:
                        nc.tensor.matmul(
                            dst, w1h[lo:hi, :], h_cur[s][lo:hi, :],
                            start=True, stop=False, tile_position=tp,
                        )
                    nc.tensor.matmul(
                        dst, w1x[lo:hi, :], xk[s][lo:hi, :],
                        start=(k == 0), stop=True, tile_position=tp,
                    )

            def g1_op(s):
                # s0 on ACT; s1 alternates ACT/DVE by step parity to
                # balance the two engines' per-step load
                g1[s] = wk.tile([128, HB], F16, name=f"g1{s}", tag=f"g1{s}")
                if s == 0 or k % 2 == 0:
                    nc.scalar.activation(g1[s][:], psg[s][:], AF.Relu, bias=gb1[:])
                else:
                    nc.vector.tensor_scalar(
                        g1[s][:], psg[s][:], gb1[:], 0.0, ALU.add, ALU.max
                    )

            def diff2_mm(s):
                pss[s] = pps.tile([128, HB], F32, name=f"pss{s}", tag="pps")
                nc.tensor.matmul(
                    pss[s][:, :], gw2[:, :], g1[s][:, :], start=True, stop=True
                )

            def sig_tt(s):
                sg[s] = wk.tile([128, HB], F16, name=f"sg{s}", tag=f"sg{s}")
                nc.scalar.activation(sg[s][:], pss[s][:], AF.Sigmoid, bias=gb2[:])
                tt[s] = wk.tile([128, HB], F16, name=f"tt{s}", tag=f"tt{s}")
                nc.gpsimd.tensor_mul(tt[s][:], sg[s][:], zk[s][:])

            def relu1(s):
                z1[s] = wk.tile([128, SB], F16, name=f"z1{s}", tag=f"z1{s}")
                nc.scalar.activation(
                    z1[s][:, 0:HB], ps1[s][0][:], AF.Relu, bias=b1[:]
                )
                nc.vector.tensor_scalar(
                    z1[s][:, HB:], ps1[s][1][:], b1[:], 0.0, ALU.add, ALU.max
                )

            def drift2_mm(s):
                ps2[s] = [
                    ppb.tile([128, HB], F32, name=f"ps2{s}{j}", tag="ppb")
                    for j in range(2)
                ]
                for j in range(2):
                    nc.tensor.matmul(
                        ps2[s][j][:, :], w2[:, :], z1[s][:, j * HB : (j + 1) * HB],
                        start=True, stop=True,
                    )

            def relu2(s):
                z2[s] = wk.tile([128, SB], F16, name=f"z2{s}", tag=f"z2{s}")
                nc.scalar.activation(
                    z2[s][:, 0:HB], ps2[s][0][:], AF.Relu, bias=b2[:]
                )
                nc.vector.tensor_scalar(
                    z2[s][:, HB:], ps2[s][1][:], b2[:], 0.0, ALU.add, ALU.max
                )

            def out_mm(s):
                ps3[s] = pps.tile([128, HB], F32, name=f"ps3{s}", tag="pps")
                if k > 0:
                    nc.tensor.matmul(
                        ps3[s][:, :], idn[:, :], h_cur[s][:, :],
                        start=True, stop=False, skip_group_check=True,
                    )
                nc.tensor.matmul(
                    ps3[s][0:64, :], w3k, z2[s][:, 0:HB],
                    start=(k == 0), stop=False, skip_group_check=True,
                )
                nc.tensor.matmul(
                    ps3[s][64:128, :], w3k, z2[s][:, HB:],
                    start=(k == 0), stop=True, tile_position=(0, 64),
                    skip_group_check=True,
                )

            def h_upd(s):
                h_new = hp.tile([128, HB], F16, name=f"h{s}", tag=f"h{s}")
                nc.vector.scalar_tensor_tensor(
                    h_new[:], ps3[s][:], dtb3[:, k : k + 1], tt[s][:],
                    ALU.add, ALU.add,
                )
                h_cur[s] = h_new

            if k == 0:
                for s in range(2):
                    tt[s] = zk[s]
                    drift1_mm(s)
                for s in range(2):
                    relu1(s)
                    drift2_mm(s)
                for s in range(2):
                    relu2(s)
                    out_mm(s)
                for s in range(2):
                    h_upd(s)
                return

            # stream 0 front: its h is ready first
            diff1_mm(0)
            drift1_mm(0)
            g1_op(0)
            diff1_mm(1)
            drift1_mm(1)
            diff2_mm(0)
            sig_tt(0)
            relu1(0)
            g1_op(1)
            drift2_mm(0)
            diff2_mm(1)
            relu2(0)
            sig_tt(1)
            relu1(1)
            out_mm(0)
            drift2_mm(1)
            h_upd(0)
            relu2(1)
            out_mm(1)
            h_upd(1)

        for k in range(STEPS):
            step_pair(k)

        # ---- readout: out = relu(h @ rW1 + rb1) @ rW2 + rb2 ----
        # per stream: both 512-row halves land col-tiled in ONE psum tile
        # (partitions 0:32 half 0, 32:64 half 1) -> one relu per stream
        osb = wk.tile([2, BC], F32, name="osb", tag="osb")

        def ro(s):
            psr = pps.tile([128, HB], F32, name="psr", tag="pps")
            for j, tp in ((0, None), (1, (64, 32))):
                lo, hi = 64 * j, 64 * (j + 1)
                nc.tensor.matmul(
                    psr[32 * j : 32 * (j + 1), :], rw1[lo:hi, :],
                    h_cur[s][lo:hi, :],
                    start=True, stop=True, tile_position=tp,
                )
            r1 = wk.tile([64, HB], F16, name=f"r1{s}", tag=f"r1{s}")
            rb1d = cf[0:64, 24:25]
            if s == 0:
                nc.scalar.activation(r1[:, :], psr[0:64, :], AF.Relu, bias=rb1d)
            else:
                nc.vector.tensor_scalar(
                    r1[:, :], psr[0:64, :], rb1d, 0.0, ALU.add, ALU.max
                )
            for j in range(2):
                sl = slice(s * SB + j * HB, s * SB + (j + 1) * HB)
                pso = pps.tile([128, HB], F32, name="pso", tag="pps")
                nc.tensor.matmul(
                    pso[0:2, :], rw2[32 * j : 32 * (j + 1), :],
                    r1[32 * j : 32 * (j + 1), :],
                    start=True, stop=True, tile_position=(32 * j, 0),
                )
                if (s + j) % 2 == 0:
                    nc.scalar.activation(
                        osb[:, sl], pso[0:2, :], AF.Identity, bias=rb2[:]
                    )
                else:
                    nc.vector.tensor_scalar(
                        osb[:, sl], pso[0:2, :], rb2[:], 0.0, ALU.add, ALU.add
                    )
                nc.sync.dma_start(d_out[:, sl], osb[:, sl])

        ro(0)
        ro(1)

    nc.compile()
    _CACHE["nc"] = nc
    return nc


def _dup(a, dt=NPF16):
    return np.ascontiguousarray(np.concatenate([a, a], axis=0).astype(dt))


def _blkdiag(a, dt=NPF16):
    n, m = a.shape
    out = np.zeros((2 * n, 2 * m), np.float32)
    out[:n, :m] = a
    out[n:, m:] = a
    return np.ascontiguousarray(out.astype(dt))


def _prep_in_maps(inputs):
    xp = np.asarray(inputs["x_path"], dtype=np.float32)
    t_span = np.asarray(inputs["t_span"], dtype=np.float32)
    dw = np.asarray(inputs["dW"], dtype=np.float32)

    Tm1 = np.int32(xp.shape[1] - 1)
    t_max = t_span[-1]
    idx = np.clip(
        (t_span[:-1] / t_max * np.float32(Tm1)).astype(np.int32), 0, Tm1
    )
    dts = (t_span[1:] - t_span[:-1]).astype(np.float32)
    sq = np.sqrt(dts).astype(np.float32)

    gscale = np.asarray(inputs["gscale"], dtype=np.float32)
    w1 = np.asarray(inputs["dW1"], dtype=np.float32)
    w2 = np.asarray(inputs["dW2"], dtype=np.float32)
    w3 = np.asarray(inputs["dW3"], dtype=np.float32)
    db1 = np.asarray(inputs["db1"], dtype=np.float32)
    db2 = np.asarray(inputs["db2"], dtype=np.float32)
    db3 = np.asarray(inputs["db3"], dtype=np.float32)
    gw1 = np.asarray(inputs["gW1"], dtype=np.float32)
    gw2 = np.asarray(inputs["gW2"], dtype=np.float32)
    gb1 = np.asarray(inputs["gb1"], dtype=np.float32)
    gb2 = np.asarray(inputs["gb2"], dtype=np.float32)
    rw1 = np.asarray(inputs["rW1"], dtype=np.float32)
    rb1 = np.asarray(inputs["rb1"], dtype=np.float32)
    rw2 = np.asarray(inputs["rW2"], dtype=np.float32)
    rb2 = np.asarray(inputs["rb2"], dtype=np.float32)

    w3s = w3[None, :, :] * dts[:, None, None]  # [STEPS, DW, H]

    def pad128(a):
        out = np.zeros((128, a.shape[1]), a.dtype)
        out[: a.shape[0]] = a
        return out

    cwe_pack = np.concatenate(
        [
            _dup(w1[H:]),  # w1x
            w2.astype(NPF16),  # w2
            w3s[0].astype(NPF16),  # w3s step 0
        ],
        axis=1,
    )
    w3s_flat = w3s[1:].transpose(1, 0, 2).reshape(DW, (STEPS - 1) * H)
    cwm_pack = np.concatenate(
        [
            _dup(w1[:H]),  # w1h
            w3s_flat.astype(NPF16),  # w3s steps 1..19
            _blkdiag(gw1),  # gw1
            _blkdiag(gw2),  # gw2
            np.eye(DW, dtype=NPF16),  # ident
            _dup(rw1),  # rw1
            pad128(_dup(rw2)),  # rw2 stacked twice (row-tiled readout)
        ],
        axis=1,
    )
    cf_pack = np.concatenate(
        [
            db1.reshape(DW, 1),
            db2.reshape(DW, 1),
            _dup((dts[:, None] * db3[None, :]).T, np.float32),
            _dup(gb1.reshape(H, 1), np.float32),
            _dup(gb2.reshape(H, 1), np.float32),
            pad128(_dup(rb1.reshape(32, 1), np.float32)),
            pad128(rb2.reshape(2, 1)),
        ],
        axis=1,
    ).astype(np.float32)

    common = {
        "cwm": np.ascontiguousarray(cwm_pack),
        "cf": np.ascontiguousarray(cf_pack),
    }

    xg = xp[:, idx, :]  # [B, STEPS, F]
    # noise pre-scale; step 0's constant sigmoid folded in (h_0 = 0)
    g1c = np.maximum(gb1, 0.0)
    sg0 = 1.0 / (1.0 + np.exp(-(g1c @ gw2 + gb2)))
    zsc = gscale[None, :] * sq[:, None]  # [STEPS, F]
    zsc[0] *= sg0

    in_maps = []
    for c in range(NCORES):
        rows = slice(c * BC, (c + 1) * BC)
        # (stream, half, b', k, f) -> (k, stream, half, f, b')
        xt = np.ascontiguousarray(
            xg[rows]
            .reshape(2, 2, HB, STEPS, FX)
            .transpose(3, 1, 4, 0, 2)
            .reshape(STEPS, 128, 2, HB)
            .astype(NPF16)
        )
        zc = dw[:, rows, :] * zsc[:, None, :]  # [STEPS, BC, H]
        zst = np.ascontiguousarray(
            zc.reshape(STEPS, 2, 2, HB, H)
            .transpose(0, 2, 4, 1, 3)
            .reshape(STEPS, 128, 2, HB)
            .astype(NPF16)
        )
        m = dict(common)
        m["xt"] = xt
        m["zst"] = zst
        m["pre"] = np.ascontiguousarray(
            np.concatenate(
                [cwe_pack, xt[0].reshape(128, SB), zst[0].reshape(128, SB)],
                axis=1,
            )
        )
        in_maps.append(m)
    return in_maps


def kernel(**inputs):
    nc = _build()
    in_maps = _prep_in_maps(inputs)
    run_kwargs = dict(_CACHE.get("run_kwargs", {}))
    res = run_bass_kernel_spmd(nc, in_maps, list(range(NCORES)), **run_kwargs)
    _CACHE["last_results"] = res
    mu = np.concatenate([res.results[c]["out"][0] for c in range(NCORES)])
    ls = np.concatenate([res.results[c]["out"][1] for c in range(NCORES)])
    return mu, ls
